# revision 10
# baseline (speedup 1.0000x reference)
"""Single-head cross-attention kernel for Trainium2, sharded across 8 NeuronCores.

Strategy (per core c):
  - Host feeds pre-transposed, pre-split shards: x1T as fp16 hi + bf16 lo,
    x2T as fp16 only, weights pre-cast to fp16 in [P, DP, D] tile layout.
    The bf16 lo-weight for Wq is derived on-chip (DVE). No device-side
    transposes, no cast DMAs.
  - Projections: QT 2-pass (hi fp16 + lo bf16), KT and V single fp16 pass
    (host-simulated rel err 5.0e-3 vs the 2e-2 gate; Q-side rounding is what
    flips argmaxes, K-side is benign). K and V shards AllGathered in fp16 as
    soon as produced; stores + triggers ride the gpsimd queue.
  - Scores TRANSPOSED, full query width: ST[keys, 512q] = KT.T-contr @ QT,
    streaming gathered K one rank-block at a time (read exactly once).
    Running max on DVE; cross-partition max via PE transpose + rank-1
    broadcast matmul.
  - PT = exp((ST - max)*scale) fp16 streams tile-by-tile into AV: two
    d-chunk passes (cols 0:512, 512:1024), 4 PSUM banks each, V blocks read
    once per chunk. Row sums ride along as N=1 matmuls into per-block [P,1]
    PSUM tiles; output scaled by 1/rowsum on PSUM eviction.
"""
import numpy as np
import ml_dtypes

import concourse.bacc as bacc
import concourse.mybir as mybir
import concourse.tile as tile
from concourse.bass_utils import run_bass_kernel_spmd
from concourse.masks import make_identity

P = 128
D = 1024            # d_in = d_kq = d_v
DP = D // P         # 8 partition tiles of the feature dim
S = 4096            # full sequence length (both x_1 and x_2)
NCORES = 8
SQ = S // NCORES    # 512 query rows per core
SK = S // NCORES    # 512 key rows per core
KPB = SK // P       # 4 key tiles per rank block
NKT = S // P        # 32 key tiles of 128
MQ = SQ // P        # 4 query row-blocks
DV2 = D // 2        # 512-wide d chunk per AV pass
SCALE = float(1.0 / np.sqrt(np.float32(D)))  # 0.03125 exactly

F32 = mybir.dt.float32
F16 = mybir.dt.float16
BF16 = mybir.dt.bfloat16
AX = mybir.AxisListType
AF = mybir.ActivationFunctionType

_CACHED_NC = None


def build_nc():
    nc = bacc.Bacc("TRN2", target_bir_lowering=False, debug=False,
                   num_devices=NCORES)
    x1h_d = nc.dram_tensor("x1h", [D, SQ], F16, kind="ExternalInput").ap()
    x1l_d = nc.dram_tensor("x1l", [D, SQ], BF16, kind="ExternalInput").ap()
    x2h_d = nc.dram_tensor("x2h", [D, SK], F16, kind="ExternalInput").ap()
    wqh_d = nc.dram_tensor("wqh", [P, DP, D], F16, kind="ExternalInput").ap()
    wkh_d = nc.dram_tensor("wkh", [P, DP, D], F16, kind="ExternalInput").ap()
    wvh_d = nc.dram_tensor("wvh", [P, DP, D], F16, kind="ExternalInput").ap()
    out = nc.dram_tensor("out", [SQ, D], F32, kind="ExternalOutput").ap()

    with tile.TileContext(nc) as tc:
        with tc.tile_pool(name="long", bufs=1) as long_pool, \
             tc.tile_pool(name="dram", bufs=1, space="DRAM") as dram:
            # warm-up operands first so the PE can start at ~1us
            dummy16 = long_pool.tile([P, P], F16, name="dummy16")
            nc.vector.memset(dummy16, 0.0)
            dummyr = long_pool.tile([P, DV2], F16, name="dummyr")
            nc.vector.memset(dummyr, 0.0)
            ones1 = long_pool.tile([1, P], F32, name="ones1")
            nc.vector.memset(ones1, 1.0)
            ones16 = long_pool.tile([P, 1], F16, name="ones16")
            nc.vector.memset(ones16, 1.0)
            ident = long_pool.tile([P, P], F32, name="ident")
            make_identity(nc, ident)

            ag_in_k = dram.tile([DP, P, SK], F16, name="ag_in_k")
            ag_out_k = dram.tile([NCORES, DP, P, SK], F16,
                                 addr_space="Shared", name="ag_out_k")
            # V AG layout: index = dvc*KPB + kt so each AV d-chunk pass reads
            # a contiguous run of tiles per rank
            ag_in_v = dram.tile([2 * KPB, P, DV2], F16, name="ag_in_v")
            ag_out_v = dram.tile([NCORES, 2 * KPB, P, DV2], F16,
                                 addr_space="Shared", name="ag_out_v")

            qt16 = [long_pool.tile([P, SQ], F16, name=f"qt16_{d}")
                    for d in range(DP)]

            with tc.tile_pool(name="wpool", bufs=1) as wp, \
                 tc.tile_pool(name="proj_sb", bufs=1) as sp, \
                 tc.tile_pool(name="proj_ps", bufs=1, space="PSUM") as pps:
                # PE warm-up at t~1us: HAM un-throttles after ~3.4us of
                # sustained activity; no input dependencies.
                for w in range(12):
                    wps = pps.tile([P, DV2], F32, tag="pp", bufs=4,
                                   name=f"warm{w}")
                    nc.tensor.matmul(wps, lhsT=dummy16, rhs=dummyr,
                                     start=True, stop=True)

                # input DMAs. sync: x2h then x1h/x1l. scalar: wkh then wqh.
                # wvh on the gpsimd SWDGE queue (free until the K stores).
                x2h_t, x1h_t, x1l_t = [], [], []
                for ki in range(DP):
                    t = sp.tile([P, SK], F16, tag="x2h", bufs=DP,
                                name=f"x2h{ki}")
                    nc.sync.dma_start(t, x2h_d[ki * P:(ki + 1) * P, :])
                    x2h_t.append(t)

                wkh_t = wp.tile([P, DP, D], F16, name="wkh")
                nc.scalar.dma_start(wkh_t, wkh_d)
                wqh_t = wp.tile([P, DP, D], F16, name="wqh")
                nc.scalar.dma_start(wqh_t, wqh_d)
                wvh_t = wp.tile([P, DP, D], F16, name="wvh")
                nc.scalar.dma_start(wvh_t, wvh_d)

                for ki in range(DP):
                    t = sp.tile([P, SQ], F16, tag="x1h", bufs=DP,
                                name=f"x1h{ki}")
                    nc.sync.dma_start(t, x1h_d[ki * P:(ki + 1) * P, :])
                    x1h_t.append(t)
                for ki in range(DP):
                    t = sp.tile([P, SQ], BF16, tag="x1l", bufs=DP,
                                name=f"x1l{ki}")
                    nc.sync.dma_start(t, x1l_d[ki * P:(ki + 1) * P, :])
                    x1l_t.append(t)

                # bf16 lo-weight for Q derived on-chip (DVE is idle here)
                wqlb_t = wp.tile([P, DP, D], BF16, name="wqlb")
                nc.vector.tensor_copy(wqlb_t, wqh_t)

                # KT projection, single fp16 pass: KT[do] = Wk[:,do].T @ x2^T
                for do in range(DP):
                    ps = pps.tile([P, SK], F32, tag="pp", bufs=4,
                                  name=f"ktps{do}")
                    cs = slice(do * P, (do + 1) * P)
                    for ki in range(DP):
                        nc.tensor.matmul(ps, lhsT=wkh_t[:, ki, cs],
                                         rhs=x2h_t[ki],
                                         start=(ki == 0), stop=(ki == DP - 1))
                    kt_t = sp.tile([P, SK], F16, tag="kt16", bufs=3,
                                   name=f"kt16_{do}")
                    nc.scalar.copy(kt_t, ps)
                    nc.gpsimd.dma_start(ag_in_k[do], kt_t)
                nc.gpsimd.collective_compute(
                    "AllGather", mybir.AluOpType.bypass,
                    replica_groups=[list(range(NCORES))],
                    ins=[ag_in_k.opt()], outs=[ag_out_k.opt()])

                # QT projection, 2-pass hi/lo
                for do in range(DP):
                    ps = pps.tile([P, SQ], F32, tag="pp", bufs=4,
                                  name=f"qtps{do}")
                    cs = slice(do * P, (do + 1) * P)
                    for ki in range(DP):
                        nc.tensor.matmul(ps, lhsT=wqh_t[:, ki, cs],
                                         rhs=x1h_t[ki],
                                         start=(ki == 0), stop=False)
                    for ki in range(DP):
                        nc.tensor.matmul(ps, lhsT=wqlb_t[:, ki, cs],
                                         rhs=x1l_t[ki],
                                         start=False, stop=(ki == DP - 1))
                    nc.scalar.copy(qt16[do], ps)

                # V projection: V[kt block] = x2 @ Wv, fp16 single pass
                for kt in range(KPB):
                    for dvc in range(2):
                        ps = pps.tile([P, DV2], F32, tag="pp", bufs=4,
                                      name=f"vps{kt}_{dvc}")
                        ds_ = slice(dvc * DV2, (dvc + 1) * DV2)
                        for ki in range(DP):
                            nc.tensor.matmul(
                                ps, lhsT=x2h_t[ki][:, kt * P:(kt + 1) * P],
                                rhs=wvh_t[:, ki, ds_],
                                start=(ki == 0), stop=(ki == DP - 1))
                        v_t = sp.tile([P, DV2], F16, tag="v16", bufs=3,
                                      name=f"v16_{kt}_{dvc}")
                        nc.scalar.copy(v_t, ps)
                        nc.gpsimd.dma_start(ag_in_v[dvc * KPB + kt], v_t)
                nc.gpsimd.collective_compute(
                    "AllGather", mybir.AluOpType.bypass,
                    replica_groups=[list(range(NCORES))],
                    ins=[ag_in_v.opt()], outs=[ag_out_v.opt()])

                # warm-keepers: the PE would otherwise idle ~25us waiting for
                # AG-K, dropping the HAM clock gate to 1.2GHz for the first
                # ~40us of the scores phase. These no-dependency matmuls keep
                # the activity window hot until gathered K arrives.
                for w in range(72):
                    wps = pps.tile([P, DV2], F32, tag="pp", bufs=4,
                                   name=f"keep{w}")
                    nc.tensor.matmul(wps, lhsT=dummy16, rhs=dummyr,
                                     start=True, stop=True)

            # ---- attention: scores -> softmax -> AV, full query width ----
            with tc.tile_pool(name="attn", bufs=1) as ap_, \
                 tc.tile_pool(name="attn_ps", bufs=1, space="PSUM") as aps:
                st_tiles = [None] * NKT
                pt_tiles = [None] * NKT
                m1 = None

                # scores, rank-block outer: each gathered K block read once
                for r in range(NCORES):
                    ktg = ap_.tile([P, DP, SK], F16, tag="ktg", bufs=2,
                                   name=f"ktg{r}")
                    for dd in range(DP):
                        # split K streaming across both HWDGE queues
                        kq = nc.sync if dd % 2 == 0 else nc.scalar
                        kq.dma_start(ktg[:, dd, :], ag_out_k[r, dd])
                    for k in range(KPB):
                        kt = r * KPB + k
                        ps = aps.tile([P, SQ], F32, tag="sc", bufs=2,
                                      name=f"stps{kt}")
                        for dd in range(DP):
                            nc.tensor.matmul(
                                ps, lhsT=ktg[:, dd, k * P:(k + 1) * P],
                                rhs=qt16[dd],
                                start=(dd == 0), stop=(dd == DP - 1))
                        st = ap_.tile([P, SQ], F32, tag="st", bufs=NKT,
                                      name=f"st{kt}")
                        nc.vector.tensor_copy(st, ps)
                        st_tiles[kt] = st
                        mn = ap_.tile([P, SQ], F32, tag="m1", bufs=3,
                                      name=f"m1_{kt}")
                        if m1 is None:
                            nc.vector.tensor_copy(mn, st)
                        else:
                            nc.vector.tensor_max(mn, m1, st)
                        m1 = mn

                # cross-partition max: PE-transpose 128-blocks, DVE reduce,
                # broadcast back with a rank-1 matmul
                mrow = ap_.tile([1, SQ], F32, name="mrow")
                for b in range(MQ):
                    tps = aps.tile([P, P], F32, tag="sc", bufs=2,
                                   name=f"tps{b}")
                    nc.tensor.transpose(tps, m1[:, b * P:(b + 1) * P], ident)
                    mq_ = ap_.tile([P, 1], F32, tag="mq", bufs=2,
                                   name=f"mq{b}")
                    nc.vector.reduce_max(mq_, tps, axis=AX.X)
                    rps = aps.tile([1, P], F32, tag="sc", bufs=2,
                                   name=f"rps{b}")
                    nc.tensor.transpose(rps, mq_, ident)
                    nc.vector.tensor_copy(mrow[:, b * P:(b + 1) * P], rps)
                mbps = aps.tile([P, SQ], F32, tag="sc", bufs=2, name="mbps")
                nc.tensor.matmul(mbps, lhsT=ones1, rhs=mrow, start=True,
                                 stop=True)
                mb = ap_.tile([P, SQ], F32, name="mb")
                nc.vector.tensor_copy(mb, mbps)

                # exp streams tile-by-tile; AV matmuls consume pt as produced.
                # f16 sub output: overflow saturates, exp -> 0, harmless.
                for kt in range(NKT):
                    tmp = ap_.tile([P, SQ], F16, tag="tmp", bufs=4,
                                   name=f"tmp{kt}")
                    nc.vector.tensor_sub(tmp, st_tiles[kt], mb)
                    pt = ap_.tile([P, SQ], F16, tag="pt", bufs=NKT,
                                  name=f"pt{kt}")
                    nc.scalar.activation(pt, tmp, AF.Exp, scale=SCALE)
                    pt_tiles[kt] = pt
                    st_tiles[kt] = None

                # AV in two d-chunk passes; V blocks read once per chunk.
                # Row sums accumulate into a single [1, SQ] PSUM tile via
                # ones.T @ PT matmuls riding pass 0; the reciprocal is taken
                # after transposing back to [P,1] blocks (full-lane DVE).
                smrow_ps = aps.tile([1, SQ], F32, tag="avs", bufs=1,
                                    name="smrow")
                rec_m = [None] * MQ
                for dvc in range(2):
                    o = [aps.tile([P, DV2], F32, tag="avo", bufs=4,
                                  name=f"avo{dvc}_{m}")
                         for m in range(MQ)]
                    for r in range(NCORES):
                        vgt = ap_.tile([P, KPB, DV2], F16, tag="vg", bufs=2,
                                       name=f"vg{dvc}_{r}")
                        # pass 0 dispatches on sync (scalar is mid-exp);
                        # pass 1 on scalar (free by then)
                        dq = nc.sync if dvc == 0 else nc.scalar
                        for k in range(KPB):
                            dq.dma_start(vgt[:, k, :],
                                         ag_out_v[r, dvc * KPB + k])
                        for k in range(KPB):
                            kt = r * KPB + k
                            first, last = (kt == 0), (kt == NKT - 1)
                            for m in range(MQ):
                                nc.tensor.matmul(
                                    o[m],
                                    lhsT=pt_tiles[kt][:, m * P:(m + 1) * P],
                                    rhs=vgt[:, k, :],
                                    start=first, stop=last)
                            if dvc == 0:
                                nc.tensor.matmul(smrow_ps, lhsT=ones16,
                                                 rhs=pt_tiles[kt],
                                                 start=first, stop=last)
                    if dvc == 0:
                        smc = ap_.tile([1, SQ], F32, name="smc")
                        nc.scalar.copy(smc, smrow_ps)
                        for m in range(MQ):
                            rtp = aps.tile([P, 1], F32, tag="sc", bufs=2,
                                           name=f"rtp{m}")
                            nc.tensor.matmul(rtp,
                                             lhsT=smc[:, m * P:(m + 1) * P],
                                             rhs=ones1[:, 0:1],
                                             start=True, stop=True)
                            smt = ap_.tile([P, 1], F32, tag="smt", bufs=2,
                                           name=f"smt{m}")
                            nc.vector.tensor_copy(smt, rtp)
                            rm = ap_.tile([P, 1], F32, tag="rm", bufs=MQ,
                                          name=f"rm{m}")
                            nc.vector.reciprocal(rm, smt)
                            rec_m[m] = rm
                    for m in range(MQ):
                        ob = ap_.tile([P, DV2], F32, tag="ob", bufs=4,
                                      name=f"ob{dvc}_{m}")
                        nc.vector.tensor_scalar_mul(ob, o[m], rec_m[m])
                        # split the tail writes across both HWDGE queues
                        oq = nc.sync if m % 2 == 0 else nc.scalar
                        oq.dma_start(
                            out[m * P:(m + 1) * P,
                                dvc * DV2:(dvc + 1) * DV2], ob)

    nc.compile()
    return nc


def make_in_maps(x_1, x_2, W_query, W_key, W_value):
    """Host-side shard prep: transpose + hi/lo split of x1, fp16 weight casts
    in the [P, DP, D] tile layout the kernel consumes."""
    f32 = np.float32
    x1t = np.ascontiguousarray(np.asarray(x_1, dtype=f32).T)
    x2t = np.ascontiguousarray(np.asarray(x_2, dtype=f32).T)

    def wtile(w):
        w = np.asarray(w, dtype=f32).reshape(DP, P, D).transpose(1, 0, 2)
        return np.ascontiguousarray(w).astype(np.float16)

    wqh = wtile(W_query)
    wkh = wtile(W_key)
    wvh = wtile(W_value)

    in_maps = []
    for c in range(NCORES):
        x1s = x1t[:, c * SQ:(c + 1) * SQ]
        x1h = x1s.astype(np.float16)
        x1l = (x1s - x1h.astype(f32)).astype(ml_dtypes.bfloat16)
        x2h = x2t[:, c * SK:(c + 1) * SK].astype(np.float16)
        in_maps.append({
            "x1h": np.ascontiguousarray(x1h),
            "x1l": np.ascontiguousarray(x1l),
            "x2h": np.ascontiguousarray(x2h),
            "wqh": wqh, "wkh": wkh, "wvh": wvh,
        })
    return in_maps


def kernel(x_1, x_2, W_query, W_key, W_value):
    global _CACHED_NC
    if _CACHED_NC is None:
        _CACHED_NC = build_nc()
    nc = _CACHED_NC
    in_maps = make_in_maps(x_1, x_2, W_query, W_key, W_value)
    res = run_bass_kernel_spmd(nc, in_maps, core_ids=list(range(NCORES)))
    return np.concatenate([res.results[c]["out"] for c in range(NCORES)],
                          axis=0)


if __name__ == "__main__":
    rng = np.random.default_rng(0)
    x1 = rng.standard_normal((S, D), dtype=np.float32)
    x2 = rng.standard_normal((S, D), dtype=np.float32)
    Wq = rng.random((D, D), dtype=np.float32)
    Wk = rng.random((D, D), dtype=np.float32)
    Wv = rng.random((D, D), dtype=np.float32)
    got = kernel(x_1=x1, x_2=x2, W_query=Wq, W_key=Wk, W_value=Wv)
    q = x1 @ Wq
    k = x2 @ Wk
    v = x2 @ Wv
    s = (q @ k.T) * np.float32(SCALE)
    s -= s.max(-1, keepdims=True)
    p = np.exp(s)
    p /= p.sum(-1, keepdims=True)
    exp = p @ v
    rel = np.linalg.norm(got - exp) / np.linalg.norm(exp)
    print("self-test rel err:", rel)


# revision 11
# speedup vs baseline: 1.0646x; 1.0646x over previous
"""Single-head cross-attention kernel for Trainium2, sharded across 8 NeuronCores.

Strategy (per core c):
  - Host feeds pre-transposed, pre-split shards: x1T as fp16 hi + bf16 lo,
    x2T as fp16 only, weights pre-cast to fp16 in [P, DP, D] tile layout.
    The bf16 lo-weight for Wq is derived on-chip (DVE). No device-side
    transposes, no cast DMAs.
  - Projections: QT 2-pass (hi fp16 + lo bf16), KT and V single fp16 pass
    (host-simulated rel err 5.0e-3 vs the 2e-2 gate; Q-side rounding is what
    flips argmaxes, K-side is benign). K and V shards AllGathered in fp16 as
    soon as produced; stores + triggers ride the gpsimd queue.
  - Scores TRANSPOSED, full query width: ST[keys, 512q] = KT.T-contr @ QT,
    streaming gathered K one rank-block at a time (read exactly once).
    Running max on DVE; cross-partition max via PE transpose + rank-1
    broadcast matmul.
  - PT = exp((ST - max)*scale) fp16 streams tile-by-tile into AV: two
    d-chunk passes (cols 0:512, 512:1024), 4 PSUM banks each, V blocks read
    once per chunk. Row sums ride along as N=1 matmuls into per-block [P,1]
    PSUM tiles; output scaled by 1/rowsum on PSUM eviction.
"""
import numpy as np
import ml_dtypes

import concourse.bacc as bacc
import concourse.mybir as mybir
import concourse.tile as tile
from concourse.bass_utils import run_bass_kernel_spmd
from concourse.masks import make_identity

P = 128
D = 1024            # d_in = d_kq = d_v
DP = D // P         # 8 partition tiles of the feature dim
S = 4096            # full sequence length (both x_1 and x_2)
NCORES = 8
SQ = S // NCORES    # 512 query rows per core
SK = S // NCORES    # 512 key rows per core
KPB = SK // P       # 4 key tiles per rank block
NKT = S // P        # 32 key tiles of 128
MQ = SQ // P        # 4 query row-blocks
DV2 = D // 2        # 512-wide d chunk per AV pass
SCALE = float(1.0 / np.sqrt(np.float32(D)))  # 0.03125 exactly

F32 = mybir.dt.float32
F16 = mybir.dt.float16
BF16 = mybir.dt.bfloat16
AX = mybir.AxisListType
AF = mybir.ActivationFunctionType

_CACHED_NC = None


def build_nc():
    nc = bacc.Bacc("TRN2", target_bir_lowering=False, debug=False,
                   num_devices=NCORES)
    x1h_d = nc.dram_tensor("x1h", [D, SQ], F16, kind="ExternalInput").ap()
    x1l_d = nc.dram_tensor("x1l", [D, SQ], BF16, kind="ExternalInput").ap()
    x2h_d = nc.dram_tensor("x2h", [D, SK], F16, kind="ExternalInput").ap()
    wqh_d = nc.dram_tensor("wqh", [P, DP, D], F16, kind="ExternalInput").ap()
    wkh_d = nc.dram_tensor("wkh", [P, DP, D], F16, kind="ExternalInput").ap()
    wvh_d = nc.dram_tensor("wvh", [P, DP, D], F16, kind="ExternalInput").ap()
    out = nc.dram_tensor("out", [SQ, D], F16, kind="ExternalOutput").ap()

    with tile.TileContext(nc) as tc:
        with tc.tile_pool(name="long", bufs=1) as long_pool, \
             tc.tile_pool(name="dram", bufs=1, space="DRAM") as dram:
            # warm-up operands first so the PE can start at ~1us
            dummy16 = long_pool.tile([P, P], F16, name="dummy16")
            nc.vector.memset(dummy16, 0.0)
            dummyr = long_pool.tile([P, DV2], F16, name="dummyr")
            nc.vector.memset(dummyr, 0.0)
            ones1 = long_pool.tile([1, P], F32, name="ones1")
            nc.vector.memset(ones1, 1.0)
            ones16 = long_pool.tile([P, 1], F16, name="ones16")
            nc.vector.memset(ones16, 1.0)
            ident = long_pool.tile([P, P], F32, name="ident")
            make_identity(nc, ident)

            ag_in_k = dram.tile([DP, P, SK], F16, name="ag_in_k")
            ag_out_k = dram.tile([NCORES, DP, P, SK], F16,
                                 addr_space="Shared", name="ag_out_k")
            # V AG layout: index = dvc*KPB + kt so each AV d-chunk pass reads
            # a contiguous run of tiles per rank
            ag_in_v = dram.tile([2 * KPB, P, DV2], F16, name="ag_in_v")
            ag_out_v = dram.tile([NCORES, 2 * KPB, P, DV2], F16,
                                 addr_space="Shared", name="ag_out_v")

            qt16 = [long_pool.tile([P, SQ], F16, name=f"qt16_{d}")
                    for d in range(DP)]

            with tc.tile_pool(name="wpool", bufs=1) as wp, \
                 tc.tile_pool(name="proj_sb", bufs=1) as sp, \
                 tc.tile_pool(name="proj_ps", bufs=1, space="PSUM") as pps:
                # PE warm-up at t~1us: HAM un-throttles after ~3.4us of
                # sustained activity; no input dependencies.
                for w in range(12):
                    wps = pps.tile([P, DV2], F32, tag="pp", bufs=4,
                                   name=f"warm{w}")
                    nc.tensor.matmul(wps, lhsT=dummy16, rhs=dummyr,
                                     start=True, stop=True)

                # input DMAs. sync: x2h then x1h/x1l. scalar: wkh then wqh.
                # wvh on the gpsimd SWDGE queue (free until the K stores).
                x2h_t, x1h_t, x1l_t = [], [], []
                for ki in range(DP):
                    t = sp.tile([P, SK], F16, tag="x2h", bufs=DP,
                                name=f"x2h{ki}")
                    nc.sync.dma_start(t, x2h_d[ki * P:(ki + 1) * P, :])
                    x2h_t.append(t)

                wkh_t = wp.tile([P, DP, D], F16, name="wkh")
                nc.scalar.dma_start(wkh_t, wkh_d)
                wqh_t = wp.tile([P, DP, D], F16, name="wqh")
                nc.scalar.dma_start(wqh_t, wqh_d)
                wvh_t = wp.tile([P, DP, D], F16, name="wvh")
                nc.scalar.dma_start(wvh_t, wvh_d)

                for ki in range(DP):
                    t = sp.tile([P, SQ], F16, tag="x1h", bufs=DP,
                                name=f"x1h{ki}")
                    nc.sync.dma_start(t, x1h_d[ki * P:(ki + 1) * P, :])
                    x1h_t.append(t)
                for ki in range(DP):
                    t = sp.tile([P, SQ], BF16, tag="x1l", bufs=DP,
                                name=f"x1l{ki}")
                    nc.sync.dma_start(t, x1l_d[ki * P:(ki + 1) * P, :])
                    x1l_t.append(t)

                # bf16 lo-weight for Q derived on-chip (DVE is idle here)
                wqlb_t = wp.tile([P, DP, D], BF16, name="wqlb")
                nc.vector.tensor_copy(wqlb_t, wqh_t)

                # KT projection, single fp16 pass: KT[do] = Wk[:,do].T @ x2^T
                for do in range(DP):
                    ps = pps.tile([P, SK], F32, tag="pp", bufs=4,
                                  name=f"ktps{do}")
                    cs = slice(do * P, (do + 1) * P)
                    for ki in range(DP):
                        nc.tensor.matmul(ps, lhsT=wkh_t[:, ki, cs],
                                         rhs=x2h_t[ki],
                                         start=(ki == 0), stop=(ki == DP - 1))
                    kt_t = sp.tile([P, SK], F16, tag="kt16", bufs=3,
                                   name=f"kt16_{do}")
                    nc.scalar.copy(kt_t, ps)
                    nc.gpsimd.dma_start(ag_in_k[do], kt_t)
                nc.gpsimd.collective_compute(
                    "AllGather", mybir.AluOpType.bypass,
                    replica_groups=[list(range(NCORES))],
                    ins=[ag_in_k.opt()], outs=[ag_out_k.opt()])

                # QT projection, 2-pass hi/lo
                for do in range(DP):
                    ps = pps.tile([P, SQ], F32, tag="pp", bufs=4,
                                  name=f"qtps{do}")
                    cs = slice(do * P, (do + 1) * P)
                    for ki in range(DP):
                        nc.tensor.matmul(ps, lhsT=wqh_t[:, ki, cs],
                                         rhs=x1h_t[ki],
                                         start=(ki == 0), stop=False)
                    for ki in range(DP):
                        nc.tensor.matmul(ps, lhsT=wqlb_t[:, ki, cs],
                                         rhs=x1l_t[ki],
                                         start=False, stop=(ki == DP - 1))
                    nc.scalar.copy(qt16[do], ps)

                # V projection: V[kt block] = x2 @ Wv, fp16 single pass
                for kt in range(KPB):
                    for dvc in range(2):
                        ps = pps.tile([P, DV2], F32, tag="pp", bufs=4,
                                      name=f"vps{kt}_{dvc}")
                        ds_ = slice(dvc * DV2, (dvc + 1) * DV2)
                        for ki in range(DP):
                            nc.tensor.matmul(
                                ps, lhsT=x2h_t[ki][:, kt * P:(kt + 1) * P],
                                rhs=wvh_t[:, ki, ds_],
                                start=(ki == 0), stop=(ki == DP - 1))
                        v_t = sp.tile([P, DV2], F16, tag="v16", bufs=3,
                                      name=f"v16_{kt}_{dvc}")
                        nc.scalar.copy(v_t, ps)
                        nc.gpsimd.dma_start(ag_in_v[dvc * KPB + kt], v_t)
                # (AG-V is triggered later, gated on scores progress, so it
                # does not contend with the first gathered-K reads)

                # warm-keepers: the PE would otherwise idle ~25us waiting for
                # AG-K, dropping the HAM clock gate to 1.2GHz for the first
                # ~40us of the scores phase. These no-dependency matmuls keep
                # the activity window hot until gathered K arrives.
                for w in range(104):
                    wps = pps.tile([P, DV2], F32, tag="pp", bufs=4,
                                   name=f"keep{w}")
                    nc.tensor.matmul(wps, lhsT=dummy16, rhs=dummyr,
                                     start=True, stop=True)

            # ---- attention: scores -> softmax -> AV, full query width ----
            with tc.tile_pool(name="attn", bufs=1) as ap_, \
                 tc.tile_pool(name="attn_ps", bufs=1, space="PSUM") as aps:
                st_tiles = [None] * NKT
                pt_tiles = [None] * NKT
                m1 = None

                # scores, rank-block outer: each gathered K block read once
                for r in range(NCORES):
                    ktg = ap_.tile([P, DP, SK], F16, tag="ktg", bufs=2,
                                   name=f"ktg{r}")
                    for dd in range(DP):
                        # split K streaming across both HWDGE queues
                        kq = nc.sync if dd % 2 == 0 else nc.scalar
                        kq.dma_start(ktg[:, dd, :], ag_out_k[r, dd])
                    for k in range(KPB):
                        kt = r * KPB + k
                        ps = aps.tile([P, SQ], F32, tag="sc", bufs=2,
                                      name=f"stps{kt}")
                        for dd in range(DP):
                            nc.tensor.matmul(
                                ps, lhsT=ktg[:, dd, k * P:(k + 1) * P],
                                rhs=qt16[dd],
                                start=(dd == 0), stop=(dd == DP - 1))
                        st = ap_.tile([P, SQ], F32, tag="st", bufs=NKT,
                                      name=f"st{kt}")
                        nc.vector.tensor_copy(st, ps)
                        st_tiles[kt] = st
                        mn = ap_.tile([P, SQ], F32, tag="m1", bufs=3,
                                      name=f"m1_{kt}")
                        if m1 is None:
                            nc.vector.tensor_copy(mn, st)
                        else:
                            nc.vector.tensor_max(mn, m1, st)
                        m1 = mn

                # AG-V trigger, delayed until scores r=1 is done (gpsimd dep
                # copy) so the collective never overlaps the first gathered-K
                # streaming reads
                agv_gate = ap_.tile([1, 4], F32, name="agv_gate")
                nc.gpsimd.tensor_copy(agv_gate, st_tiles[7][0:1, 0:4])
                nc.gpsimd.collective_compute(
                    "AllGather", mybir.AluOpType.bypass,
                    replica_groups=[list(range(NCORES))],
                    ins=[ag_in_v.opt()], outs=[ag_out_v.opt()])

                # cross-partition max: PE-transpose 128-blocks, DVE reduce,
                # broadcast back with a rank-1 matmul
                mrow = ap_.tile([1, SQ], F32, name="mrow")
                for b in range(MQ):
                    tps = aps.tile([P, P], F32, tag="sc", bufs=2,
                                   name=f"tps{b}")
                    nc.tensor.transpose(tps, m1[:, b * P:(b + 1) * P], ident)
                    mq_ = ap_.tile([P, 1], F32, tag="mq", bufs=2,
                                   name=f"mq{b}")
                    nc.vector.reduce_max(mq_, tps, axis=AX.X)
                    rps = aps.tile([1, P], F32, tag="sc", bufs=2,
                                   name=f"rps{b}")
                    nc.tensor.transpose(rps, mq_, ident)
                    nc.vector.tensor_copy(mrow[:, b * P:(b + 1) * P], rps)
                mbps = aps.tile([P, SQ], F32, tag="sc", bufs=2, name="mbps")
                nc.tensor.matmul(mbps, lhsT=ones1, rhs=mrow, start=True,
                                 stop=True)
                mb = ap_.tile([P, SQ], F32, name="mb")
                nc.vector.tensor_copy(mb, mbps)

                # exp streams tile-by-tile; AV matmuls consume pt as produced.
                # f16 sub output: overflow saturates, exp -> 0, harmless.
                for kt in range(NKT):
                    tmp = ap_.tile([P, SQ], F16, tag="tmp", bufs=4,
                                   name=f"tmp{kt}")
                    nc.vector.tensor_sub(tmp, st_tiles[kt], mb)
                    pt = ap_.tile([P, SQ], F16, tag="pt", bufs=NKT,
                                  name=f"pt{kt}")
                    nc.scalar.activation(pt, tmp, AF.Exp, scale=SCALE)
                    pt_tiles[kt] = pt
                    st_tiles[kt] = None

                # AV in two d-chunk passes; V blocks read once per chunk.
                # Row sums accumulate into a single [1, SQ] PSUM tile via
                # ones.T @ PT matmuls riding pass 0; the reciprocal is taken
                # after transposing back to [P,1] blocks (full-lane DVE).
                smrow_ps = aps.tile([1, SQ], F32, tag="avs", bufs=1,
                                    name="smrow")
                rec_m = [None] * MQ
                for dvc in range(2):
                    o = [aps.tile([P, DV2], F32, tag="avo", bufs=4,
                                  name=f"avo{dvc}_{m}")
                         for m in range(MQ)]
                    for r in range(NCORES):
                        vgt = ap_.tile([P, KPB, DV2], F16, tag="vg", bufs=2,
                                       name=f"vg{dvc}_{r}")
                        # pass 0 dispatches on sync (scalar is mid-exp);
                        # pass 1 on scalar (free by then) except r=0 which is
                        # prefetched on sync right behind pass 0
                        dq = nc.sync if (dvc == 0 or r == 0) else nc.scalar
                        for k in range(KPB):
                            dq.dma_start(vgt[:, k, :],
                                         ag_out_v[r, dvc * KPB + k])
                        for k in range(KPB):
                            kt = r * KPB + k
                            first, last = (kt == 0), (kt == NKT - 1)
                            for m in range(MQ):
                                nc.tensor.matmul(
                                    o[m],
                                    lhsT=pt_tiles[kt][:, m * P:(m + 1) * P],
                                    rhs=vgt[:, k, :],
                                    start=first, stop=last)
                            if dvc == 0:
                                nc.tensor.matmul(smrow_ps, lhsT=ones16,
                                                 rhs=pt_tiles[kt],
                                                 start=first, stop=last)
                    if dvc == 0:
                        smc = ap_.tile([1, SQ], F32, name="smc")
                        nc.scalar.copy(smc, smrow_ps)
                        for m in range(MQ):
                            rtp = aps.tile([P, 1], F32, tag="sc", bufs=2,
                                           name=f"rtp{m}")
                            nc.tensor.matmul(rtp,
                                             lhsT=smc[:, m * P:(m + 1) * P],
                                             rhs=ones1[:, 0:1],
                                             start=True, stop=True)
                            smt = ap_.tile([P, 1], F32, tag="smt", bufs=2,
                                           name=f"smt{m}")
                            nc.vector.tensor_copy(smt, rtp)
                            rm = ap_.tile([P, 1], F32, tag="rm", bufs=MQ,
                                          name=f"rm{m}")
                            nc.vector.reciprocal(rm, smt)
                            rec_m[m] = rm
                    for m in range(MQ):
                        ob = ap_.tile([P, DV2], F16, tag="ob", bufs=4,
                                      name=f"ob{dvc}_{m}")
                        nc.vector.tensor_scalar_mul(ob, o[m], rec_m[m])
                        # split the tail writes across both HWDGE queues
                        oq = nc.sync if m % 2 == 0 else nc.scalar
                        oq.dma_start(
                            out[m * P:(m + 1) * P,
                                dvc * DV2:(dvc + 1) * DV2], ob)

    nc.compile()
    return nc


def make_in_maps(x_1, x_2, W_query, W_key, W_value):
    """Host-side shard prep: transpose + hi/lo split of x1, fp16 weight casts
    in the [P, DP, D] tile layout the kernel consumes."""
    f32 = np.float32
    x1t = np.ascontiguousarray(np.asarray(x_1, dtype=f32).T)
    x2t = np.ascontiguousarray(np.asarray(x_2, dtype=f32).T)

    def wtile(w):
        w = np.asarray(w, dtype=f32).reshape(DP, P, D).transpose(1, 0, 2)
        return np.ascontiguousarray(w).astype(np.float16)

    wqh = wtile(W_query)
    wkh = wtile(W_key)
    wvh = wtile(W_value)

    in_maps = []
    for c in range(NCORES):
        x1s = x1t[:, c * SQ:(c + 1) * SQ]
        x1h = x1s.astype(np.float16)
        x1l = (x1s - x1h.astype(f32)).astype(ml_dtypes.bfloat16)
        x2h = x2t[:, c * SK:(c + 1) * SK].astype(np.float16)
        in_maps.append({
            "x1h": np.ascontiguousarray(x1h),
            "x1l": np.ascontiguousarray(x1l),
            "x2h": np.ascontiguousarray(x2h),
            "wqh": wqh, "wkh": wkh, "wvh": wvh,
        })
    return in_maps


def kernel(x_1, x_2, W_query, W_key, W_value):
    global _CACHED_NC
    if _CACHED_NC is None:
        _CACHED_NC = build_nc()
    nc = _CACHED_NC
    in_maps = make_in_maps(x_1, x_2, W_query, W_key, W_value)
    res = run_bass_kernel_spmd(nc, in_maps, core_ids=list(range(NCORES)))
    return np.concatenate([res.results[c]["out"] for c in range(NCORES)],
                          axis=0).astype(np.float32)


if __name__ == "__main__":
    rng = np.random.default_rng(0)
    x1 = rng.standard_normal((S, D), dtype=np.float32)
    x2 = rng.standard_normal((S, D), dtype=np.float32)
    Wq = rng.random((D, D), dtype=np.float32)
    Wk = rng.random((D, D), dtype=np.float32)
    Wv = rng.random((D, D), dtype=np.float32)
    got = kernel(x_1=x1, x_2=x2, W_query=Wq, W_key=Wk, W_value=Wv)
    q = x1 @ Wq
    k = x2 @ Wk
    v = x2 @ Wv
    s = (q @ k.T) * np.float32(SCALE)
    s -= s.max(-1, keepdims=True)
    p = np.exp(s)
    p /= p.sum(-1, keepdims=True)
    exp = p @ v
    rel = np.linalg.norm(got - exp) / np.linalg.norm(exp)
    print("self-test rel err:", rel)
